# revision 2
# baseline (speedup 1.0000x reference)
"""Trainium2 Bass kernel for a 2-layer tanh RNN (B=64, T=512, E=H=512) with
ragged sequence lengths.

Strategy: data-parallel over batch (8 rows/core on 8 cores). Per core:
  - indirect-DMA embedding gather + PE transpose into e-major layout
  - bulk input matmul for layer 0 (PE-efficient, amortized weight loads)
  - 512-step recurrent scan for layer 0 with W_hh stationary; hidden state
    kept transposed [H(4x128 partitions), batch(8 cols)] so the elementwise
    chain uses all 128 lanes and the next step's matmul needs no transpose
  - bulk input matmul for layer 1 from the stored h0 sequence (already
    transposed in SBUF)
  - 512-step scan for layer 1
  - both hidden sequences stream to DRAM; the host extracts each row's
    final state at t = length-1 (masked rows freeze there; later columns
    are computed but never read)
"""

import numpy as np

import concourse.bass as bass
import concourse.mybir as mybir
import concourse.tile as tile
from concourse.bass_utils import run_bass_kernel_spmd
from concourse.masks import make_identity

F32 = mybir.dt.float32
I32 = mybir.dt.int32

B, T, V, E, H, NLAYER = 64, 512, 32000, 512, 512, 2
NCORES = 8
BL = B // NCORES          # 8 batch rows per core
KC = H // 128             # 4 contraction chunks
MC = H // 128             # 4 output-row tiles
NCOLS = T * BL            # 4096 (t-major, b-minor) columns per core
NBLK = NCOLS // 512       # 8 column blocks for bulk matmuls


def _split_multi_waits(nc):
    """This container's walrus supports a single sync-wait per instruction;
    hoist extra waits onto standalone EventSemaphore (wait) ops just before
    the instruction on the same engine."""
    wid = 0
    for f in nc.m.functions:
        for bb in f.blocks:
            newl = []
            for ins in bb.instructions:
                si = ins.sync_info
                waits = list(si.on_wait) if si and si.on_wait else []
                if len(waits) > 1:
                    for w in waits[:-1]:
                        nop = mybir.InstEventSemaphore(
                            name=f"WSPLIT-{wid}", ins=[], outs=[]
                        )
                        wid += 1
                        nop.engine = ins.engine
                        nop.sync_info = mybir.SyncInfo(on_wait=[w], on_update=[])
                        newl.append(nop)
                    si.on_wait = [waits[-1]]
                    ins.sync_info = si
                newl.append(ins)
            bb.instructions[:] = newl


def _build_nc():
    nc = bass.Bass()

    emb_d = nc.dram_tensor("emb", [V, E], F32, kind="ExternalInput")
    wih0_d = nc.dram_tensor("wih0T", [E, H], F32, kind="ExternalInput")
    wih1_d = nc.dram_tensor("wih1T", [H, H], F32, kind="ExternalInput")
    whh0_d = nc.dram_tensor("whh0T", [H, H], F32, kind="ExternalInput")
    whh1_d = nc.dram_tensor("whh1T", [H, H], F32, kind="ExternalInput")
    bias0_d = nc.dram_tensor("bias0", [128, MC], F32, kind="ExternalInput")
    bias1_d = nc.dram_tensor("bias1", [128, MC], F32, kind="ExternalInput")
    tok_d = nc.dram_tensor("tok", [128, NCOLS // 128], I32, kind="ExternalInput")
    h0seq_d = nc.dram_tensor("h0seq", [MC, 128, NCOLS], F32, kind="ExternalOutput")
    h1seq_d = nc.dram_tensor("h1seq", [MC, 128, NCOLS], F32, kind="ExternalOutput")

    NGATHER = NCOLS // 128  # 32

    with tile.TileContext(nc) as tc:
        with (
            tc.tile_pool(name="const", bufs=1) as cpool,
            tc.tile_pool(name="big", bufs=KC) as bigpool,
            tc.tile_pool(name="pre", bufs=MC) as prepool,
            tc.tile_pool(name="gat", bufs=3) as gatpool,
            tc.tile_pool(name="psA", bufs=3, space="PSUM") as psA,
            tc.tile_pool(name="psT", bufs=2, space="PSUM") as psT,
            tc.tile_pool(name="psS", bufs=2, space="PSUM") as psS,
        ):
            # ---- constants into SBUF ----
            def load_w(dram):
                sb = cpool.tile([128, KC * H], F32, tag=f"w_{dram.name}", name=f"w_{dram.name}")
                for k in range(KC):
                    nc.sync.dma_start(
                        sb[:, k * H : (k + 1) * H], dram[k * 128 : (k + 1) * 128, :]
                    )
                return sb

            wih0_sb = load_w(wih0_d)
            wih1_sb = load_w(wih1_d)
            whh0_sb = load_w(whh0_d)
            whh1_sb = load_w(whh1_d)
            bias0_sb = cpool.tile([128, MC], F32, tag="b0")
            nc.sync.dma_start(bias0_sb[:], bias0_d[:])
            bias1_sb = cpool.tile([128, MC], F32, tag="b1")
            nc.sync.dma_start(bias1_sb[:], bias1_d[:])
            idx_sb = cpool.tile([128, NGATHER], I32, tag="idx")
            nc.sync.dma_start(idx_sb[:], tok_d[:])
            ident = cpool.tile([128, 128], F32, tag="ident")
            make_identity(nc, ident[:])

            def wtile(sb, k, m):
                return sb[:, k * H + m * 128 : k * H + m * 128 + 128]

            # ---- embedding gather + transpose to e-major ----
            xeT = [bigpool.tile([128, NCOLS], F32, tag="big", name=f"xeT{k}") for k in range(KC)]
            for g in range(NGATHER):
                gt = gatpool.tile([128, E], F32, tag="gt", name=f"gt{g}")
                nc.gpsimd.indirect_dma_start(
                    out=gt[:],
                    out_offset=None,
                    in_=emb_d[:],
                    in_offset=bass.IndirectOffsetOnAxis(ap=idx_sb[:, g : g + 1], axis=0),
                )
                for k in range(KC):
                    pst = psT.tile([128, 128], F32, tag="pst", name=f"pst{g}_{k}")
                    nc.tensor.transpose(pst[:], gt[:, k * 128 : (k + 1) * 128], ident[:])
                    nc.vector.tensor_copy(xeT[k][:, g * 128 : (g + 1) * 128], pst[:])

            # ---- bulk input matmul (shared by phase A and C) ----
            def bulk_input(w_sb, rhs_tiles, bias_sb, out_tiles):
                for m in range(MC):
                    for n in range(NBLK):
                        ps = psA.tile([128, 512], F32, tag="psA", name=f"psA_{m}_{n}")
                        for k in range(KC):
                            nc.tensor.matmul(
                                ps[:],
                                wtile(w_sb, k, m),
                                rhs_tiles[k][:, n * 512 : (n + 1) * 512],
                                start=(k == 0),
                                stop=(k == KC - 1),
                            )
                        nc.vector.tensor_scalar_add(
                            out_tiles[m][:, n * 512 : (n + 1) * 512],
                            ps[:],
                            bias_sb[:, m : m + 1],
                        )

            # ---- recurrent scan (shared by phase B and D) ----
            def scan(w_sb, pre_tiles, seq_tiles):
                for t in range(T):
                    if t == 0:
                        for m in range(MC):
                            nc.scalar.activation(
                                seq_tiles[m][:, 0:BL],
                                pre_tiles[m][:, 0:BL],
                                mybir.ActivationFunctionType.Tanh,
                            )
                        continue
                    ps = psS.tile([128, MC * BL], F32, tag="psS", name=f"psS_{t}")
                    for m in range(MC):
                        for k in range(KC):
                            nc.tensor.matmul(
                                ps[:, m * BL : (m + 1) * BL],
                                wtile(w_sb, k, m),
                                seq_tiles[k][:, (t - 1) * BL : t * BL],
                                start=(k == 0),
                                stop=(k == KC - 1),
                            )
                    for m in range(MC):
                        nc.vector.tensor_add(
                            ps[:, m * BL : (m + 1) * BL],
                            ps[:, m * BL : (m + 1) * BL],
                            pre_tiles[m][:, t * BL : (t + 1) * BL],
                        )
                    for m in range(MC):
                        nc.scalar.activation(
                            seq_tiles[m][:, t * BL : (t + 1) * BL],
                            ps[:, m * BL : (m + 1) * BL],
                            mybir.ActivationFunctionType.Tanh,
                        )

            # Phase A: pre0 = W_ih0 @ xe + (b_ih0 + b_hh0)
            pre0T = [prepool.tile([128, NCOLS], F32, tag="pre", name=f"pre0T{m}") for m in range(MC)]
            bulk_input(wih0_sb, xeT, bias0_sb, pre0T)

            # Phase B: layer-0 scan
            h0seqT = [bigpool.tile([128, NCOLS], F32, tag="big", name=f"h0seqT{m}") for m in range(MC)]
            scan(whh0_sb, pre0T, h0seqT)
            for m in range(MC):
                nc.sync.dma_start(h0seq_d[m], h0seqT[m][:])

            # Phase C: pre1 = W_ih1 @ h0seq + (b_ih1 + b_hh1)
            pre1T = [prepool.tile([128, NCOLS], F32, tag="pre", name=f"pre1T{m}") for m in range(MC)]
            bulk_input(wih1_sb, h0seqT, bias1_sb, pre1T)

            # Phase D: layer-1 scan
            h1seqT = [bigpool.tile([128, NCOLS], F32, tag="big", name=f"h1seqT{m}") for m in range(MC)]
            scan(whh1_sb, pre1T, h1seqT)
            for m in range(MC):
                nc.sync.dma_start(h1seq_d[m], h1seqT[m][:])

    _split_multi_waits(nc)
    return nc


_NC_CACHE = None


def _get_nc():
    global _NC_CACHE
    if _NC_CACHE is None:
        _NC_CACHE = _build_nc()
    return _NC_CACHE


def make_in_maps(x, emb, W_ih, W_hh, b_ih, b_hh):
    x = np.asarray(x).astype(np.int32)
    emb = np.ascontiguousarray(np.asarray(emb, dtype=np.float32))
    wih0T = np.ascontiguousarray(np.asarray(W_ih[0], np.float32).T)
    wih1T = np.ascontiguousarray(np.asarray(W_ih[1], np.float32).T)
    whh0T = np.ascontiguousarray(np.asarray(W_hh[0], np.float32).T)
    whh1T = np.ascontiguousarray(np.asarray(W_hh[1], np.float32).T)
    b0 = np.asarray(b_ih[0] + b_hh[0], np.float32).reshape(MC, 128).T.copy()
    b1 = np.asarray(b_ih[1] + b_hh[1], np.float32).reshape(MC, 128).T.copy()
    in_maps = []
    for c in range(NCORES):
        rows = x[c * BL : (c + 1) * BL]          # [BL, T]
        # column j = t*BL + b  ->  tok[j] = rows[b, t]
        tokj = rows.T.reshape(NCOLS)             # t-major, b-minor
        tok = tokj.reshape(NCOLS // 128, 128).T.copy()  # [128, 32] partition-major
        in_maps.append(
            {
                "emb": emb,
                "wih0T": wih0T,
                "wih1T": wih1T,
                "whh0T": whh0T,
                "whh1T": whh1T,
                "bias0": b0,
                "bias1": b1,
                "tok": tok,
            }
        )
    return in_maps


def assemble_output(results, lengths):
    lengths = np.asarray(lengths).astype(np.int64)
    hidden = np.zeros((B, NLAYER, H), np.float32)
    for c in range(NCORES):
        for li, key in enumerate(("h0seq", "h1seq")):
            seq = results[c][key]                 # [MC, 128, NCOLS]
            # seq[m, p, t*BL + b] = h[t, b, m*128 + p]
            hh = seq.reshape(MC, 128, T, BL)
            for b in range(BL):
                ell = int(lengths[c * BL + b])
                hidden[c * BL + b, li] = hh[:, :, ell - 1, b].reshape(H)
    return hidden


def kernel(x, lengths, emb, W_ih, W_hh, b_ih, b_hh):
    nc = _get_nc()
    in_maps = make_in_maps(x, emb, W_ih, W_hh, b_ih, b_hh)
    res = run_bass_kernel_spmd(nc, in_maps, core_ids=list(range(NCORES)))
    hidden = assemble_output(res.results, lengths)
    return hidden[:, -1], hidden


# revision 15
# speedup vs baseline: 196.6701x; 196.6701x over previous
"""Trainium2 Bass kernel for a 2-layer tanh RNN (B=64, T=512, E=H=512) with
ragged sequence lengths.

Strategy: data-parallel over batch (8 rows/core on 8 cores). Per core:
  - indirect-DMA embedding gather + PE transpose into e-major layout
  - bulk input matmul for layer 0 (PE-efficient, amortized weight loads)
  - 512-step recurrent scan for layer 0 with W_hh stationary; hidden state
    kept transposed [H(4x128 partitions), batch(8 cols)] so the elementwise
    chain uses all 128 lanes and the next step's matmul needs no transpose
  - bulk input matmul for layer 1 from the stored h0 sequence (already
    transposed in SBUF)
  - 512-step scan for layer 1
  - both hidden sequences stream to DRAM; the host extracts each row's
    final state at t = length-1 (masked rows freeze there; later columns
    are computed but never read)
"""

import numpy as np

import concourse.bass as bass
import concourse.mybir as mybir
import concourse.tile as tile
from concourse.bass_utils import run_bass_kernel_spmd
from concourse.masks import make_identity
from concourse.tile import add_dep_helper

F32 = mybir.dt.float32
I32 = mybir.dt.int32

B, T, V, E, H, NLAYER = 64, 512, 32000, 512, 512, 2
NCORES = 8
BL = B // NCORES          # 8 batch rows per core
KC = H // 128             # 4 contraction chunks
MC = H // 128             # 4 output-row tiles
NCOLS = T * BL            # 4096 (t-major, b-minor) columns per core
NBLK = NCOLS // 512       # 8 column blocks for bulk matmuls


def _split_multi_waits(nc):
    """This container's walrus supports a single sync-wait per instruction;
    hoist extra waits onto standalone EventSemaphore (wait) ops just before
    the instruction on the same engine."""
    wid = 0
    for f in nc.m.functions:
        for bb in f.blocks:
            newl = []
            for ins in bb.instructions:
                si = ins.sync_info
                waits = list(si.on_wait) if si and si.on_wait else []
                if len(waits) > 1:
                    for w in waits[:-1]:
                        nop = mybir.InstEventSemaphore(
                            name=f"WSPLIT-{wid}", ins=[], outs=[]
                        )
                        wid += 1
                        nop.engine = ins.engine
                        nop.sync_info = mybir.SyncInfo(on_wait=[w], on_update=[])
                        newl.append(nop)
                    si.on_wait = [waits[-1]]
                    ins.sync_info = si
                newl.append(ins)
            bb.instructions[:] = newl


def _build_nc(T_scan=T, scan_dt=mybir.dt.float16, psS_bufs=2):
    # scan_dt: optional mybir dtype (e.g. float16) for the scan-side matmul
    # operands: W_hh0/1 + W_ih1 weights, the h sequences, and the seq outputs.
    SDT = scan_dt or F32

    nc = bass.Bass()

    emb_d = nc.dram_tensor("emb", [V, E], SDT, kind="ExternalInput")
    wih0_d = nc.dram_tensor("wih0T", [E, H], SDT, kind="ExternalInput")
    wih1_d = nc.dram_tensor("wih1T", [H, H], SDT, kind="ExternalInput")
    whh0_d = nc.dram_tensor("whh0T", [H, H], SDT, kind="ExternalInput")
    whh1_d = nc.dram_tensor("whh1T", [H, H], SDT, kind="ExternalInput")
    bias0_d = nc.dram_tensor("bias0", [128, MC], F32, kind="ExternalInput")
    bias1_d = nc.dram_tensor("bias1", [128, MC], F32, kind="ExternalInput")
    tok_d = nc.dram_tensor("tok", [128, NCOLS // 128], I32, kind="ExternalInput")
    h0seq_d = nc.dram_tensor("h0seq", [MC, 128, NCOLS], SDT, kind="ExternalOutput")
    h1seq_d = nc.dram_tensor("h1seq", [MC, 128, NCOLS], SDT, kind="ExternalOutput")

    NGATHER = NCOLS // 128  # 32

    with tile.TileContext(nc) as tc:
        with (
            tc.tile_pool(name="const", bufs=1) as cpool,
            tc.tile_pool(name="big", bufs=KC) as bigpool,
            tc.tile_pool(name="pre", bufs=2) as prepool,
            tc.tile_pool(name="seq", bufs=2) as seqpool,
            tc.tile_pool(name="gat", bufs=3) as gatpool,
            tc.tile_pool(name="psA", bufs=3, space="PSUM") as psA,
            tc.tile_pool(name="psT", bufs=2, space="PSUM") as psT,
            tc.tile_pool(name="psS", bufs=psS_bufs, space="PSUM") as psS,
        ):
            # ---- constants into SBUF ----
            def load_w(dram):
                sb = cpool.tile([128, KC * H], dram.dtype, tag=f"w_{dram.name}", name=f"w_{dram.name}")
                for k in range(KC):
                    nc.sync.dma_start(
                        sb[:, k * H : (k + 1) * H], dram[k * 128 : (k + 1) * 128, :]
                    )
                return sb

            wih0_sb = load_w(wih0_d)
            wih1_sb = load_w(wih1_d)
            whh0_sb = load_w(whh0_d)
            whh1_sb = load_w(whh1_d)
            bias0_sb = cpool.tile([128, MC], F32, tag="b0")
            nc.sync.dma_start(bias0_sb[:], bias0_d[:])
            bias1_sb = cpool.tile([128, MC], F32, tag="b1")
            nc.sync.dma_start(bias1_sb[:], bias1_d[:])
            idx_sb = cpool.tile([128, NGATHER], I32, tag="idx")
            nc.sync.dma_start(idx_sb[:], tok_d[:])
            ident = cpool.tile([128, 128], SDT, tag="ident")
            make_identity(nc, ident[:])

            def wtile(sb, k, m):
                return sb[:, k * H + m * 128 : k * H + m * 128 + 128]

            # ---- embedding gather + transpose to e-major ----
            xeT = [bigpool.tile([128, NCOLS], SDT, tag="big", name=f"xeT{k}", bufs=KC) for k in range(KC)]
            for g in range(NGATHER):
                gt = gatpool.tile([128, E], SDT, tag="gt", name=f"gt{g}")
                nc.gpsimd.indirect_dma_start(
                    out=gt[:],
                    out_offset=None,
                    in_=emb_d[:],
                    in_offset=bass.IndirectOffsetOnAxis(ap=idx_sb[:, g : g + 1], axis=0),
                )
                for k in range(KC):
                    pst = psT.tile([128, 128], SDT, tag="pst", name=f"pst{g}_{k}")
                    nc.tensor.transpose(pst[:], gt[:, k * 128 : (k + 1) * 128], ident[:])
                    nc.vector.tensor_copy(xeT[k][:, g * 128 : (g + 1) * 128], pst[:])

            # ---- bulk input matmul (shared by phase A and C) ----
            def bulk_input(w_sb, rhs_view, bias_sb, out_big):
                for m in range(MC):
                    for n in range(NBLK):
                        ps = psA.tile([128, 512], F32, tag="psA", name=f"psA_{m}_{n}")
                        prev = None
                        for k in range(KC):
                            mm = nc.tensor.matmul(
                                ps[:],
                                wtile(w_sb, k, m),
                                rhs_view(k, n),
                                start=(k == 0),
                                stop=(k == KC - 1),
                            )
                            if prev is not None:
                                add_dep_helper(mm.ins, prev.ins, sync=False,
                                               reason="k-order in accum group")
                            prev = mm
                        nc.vector.tensor_scalar_add(
                            out_big[:, m * NCOLS + n * 512 : m * NCOLS + (n + 1) * 512],
                            ps[:],
                            bias_sb[:, m : m + 1],
                        )

            # ---- recurrent scan (shared by phase B and D) ----
            # seq_big/pre_big hold all four 128-row chunks side by side:
            # column k*NCOLS + t*BL + b  <->  h[t, b, k*128 + p]
            def scan(w_sb, pre_big, seq_big):
                pre3 = pre_big.rearrange("p (m n) -> p m n", m=MC)
                seq3 = seq_big.rearrange("p (k n) -> p k n", k=KC)
                for t in range(T_scan):
                    if t == 0:
                        nc.scalar.activation(
                            seq3[:, :, 0:BL],
                            pre3[:, :, 0:BL],
                            mybir.ActivationFunctionType.Tanh,
                        )
                        continue
                    ps = psS.tile([128, MC * BL], F32, tag="psS", name=f"psS_{t}")
                    # Region-major accumulation: per output region m, an
                    # identity matmul deposits pre (psum := pre), then the four
                    # W_hh chunk matmuls accumulate. Fully order-pinned: other
                    # interleavings of start=True groups miscompute on HW.
                    prev = None
                    for m in range(MC):
                        i0 = nc.tensor.matmul(
                            ps[:, m * BL : (m + 1) * BL],
                            ident[:],
                            pre_big[:, m * NCOLS + t * BL : m * NCOLS + (t + 1) * BL],
                            start=True,
                            stop=False,
                        )
                        if prev is not None:
                            add_dep_helper(i0.ins, prev.ins, sync=False,
                                           reason="step mm order")
                        prev = i0
                        for k in range(KC):
                            mm = nc.tensor.matmul(
                                ps[:, m * BL : (m + 1) * BL],
                                wtile(w_sb, k, m),
                                seq_big[:, k * NCOLS + (t - 1) * BL : k * NCOLS + t * BL],
                                start=False,
                                stop=(k == KC - 1),
                            )
                            add_dep_helper(mm.ins, prev.ins, sync=False,
                                           reason="step mm order")
                            prev = mm
                    nc.scalar.activation(
                        seq3[:, :, t * BL : (t + 1) * BL],
                        ps[:].rearrange("p (m b) -> p m b", m=MC),
                        mybir.ActivationFunctionType.Tanh,
                    )

            # Phase A: pre0 = W_ih0 @ xe + (b_ih0 + b_hh0)
            pre0_big = prepool.tile([128, MC * NCOLS], SDT, tag="pre", name="pre0_big")
            bulk_input(wih0_sb, lambda k, n: xeT[k][:, n * 512 : (n + 1) * 512],
                       bias0_sb, pre0_big)

            # Phase B: layer-0 scan
            h0_big = seqpool.tile([128, KC * NCOLS], SDT, tag="seq", name="h0_big")
            scan(whh0_sb, pre0_big, h0_big)
            for m in range(MC):
                nc.sync.dma_start(h0seq_d[m], h0_big[:, m * NCOLS : (m + 1) * NCOLS])

            # Phase C: pre1 = W_ih1 @ h0seq + (b_ih1 + b_hh1)
            pre1_big = prepool.tile([128, MC * NCOLS], SDT, tag="pre", name="pre1_big")
            bulk_input(wih1_sb, lambda k, n: h0_big[:, k * NCOLS + n * 512 : k * NCOLS + (n + 1) * 512],
                       bias1_sb, pre1_big)

            # Phase D: layer-1 scan
            h1_big = seqpool.tile([128, KC * NCOLS], SDT, tag="seq", name="h1_big")
            scan(whh1_sb, pre1_big, h1_big)
            for m in range(MC):
                nc.sync.dma_start(h1seq_d[m], h1_big[:, m * NCOLS : (m + 1) * NCOLS])

    _split_multi_waits(nc)
    return nc


_NC_CACHE = None


def _get_nc():
    global _NC_CACHE
    if _NC_CACHE is None:
        _NC_CACHE = _build_nc()
    return _NC_CACHE


SCAN_NP_DT = np.float16


def make_in_maps(x, emb, W_ih, W_hh, b_ih, b_hh, scan_np_dt=None):
    if scan_np_dt is None:
        scan_np_dt = SCAN_NP_DT
    x = np.asarray(x).astype(np.int32)
    emb = np.ascontiguousarray(np.asarray(emb, dtype=np.float32).astype(scan_np_dt))
    wih0T = np.ascontiguousarray(np.asarray(W_ih[0], np.float32).T.astype(scan_np_dt))
    wih1T = np.ascontiguousarray(np.asarray(W_ih[1], np.float32).T.astype(scan_np_dt))
    whh0T = np.ascontiguousarray(np.asarray(W_hh[0], np.float32).T.astype(scan_np_dt))
    whh1T = np.ascontiguousarray(np.asarray(W_hh[1], np.float32).T.astype(scan_np_dt))
    b0 = np.asarray(b_ih[0] + b_hh[0], np.float32).reshape(MC, 128).T.copy()
    b1 = np.asarray(b_ih[1] + b_hh[1], np.float32).reshape(MC, 128).T.copy()
    in_maps = []
    for c in range(NCORES):
        rows = x[c * BL : (c + 1) * BL]          # [BL, T]
        # column j = t*BL + b  ->  tok[j] = rows[b, t]
        tokj = rows.T.reshape(NCOLS)             # t-major, b-minor
        tok = tokj.reshape(NCOLS // 128, 128).T.copy()  # [128, 32] partition-major
        in_maps.append(
            {
                "emb": emb,
                "wih0T": wih0T,
                "wih1T": wih1T,
                "whh0T": whh0T,
                "whh1T": whh1T,
                "bias0": b0,
                "bias1": b1,
                "tok": tok,
            }
        )
    return in_maps


def assemble_output(results, lengths):
    lengths = np.asarray(lengths).astype(np.int64)
    hidden = np.zeros((B, NLAYER, H), np.float32)
    for c in range(NCORES):
        for li, key in enumerate(("h0seq", "h1seq")):
            seq = np.asarray(results[c][key], np.float32)  # [MC, 128, NCOLS]
            # seq[m, p, t*BL + b] = h[t, b, m*128 + p]
            hh = seq.reshape(MC, 128, T, BL)
            for b in range(BL):
                ell = int(lengths[c * BL + b])
                hidden[c * BL + b, li] = hh[:, :, ell - 1, b].reshape(H)
    return hidden


class _Runner:
    """Persistent PJRT executor for the SPMD kernel: compile once, reuse the
    jitted executable and donated-zero output buffers across calls."""

    def __init__(self, nc, n_cores):
        import jax
        import numpy as _np
        from jax.sharding import Mesh, PartitionSpec, NamedSharding
        from jax.experimental.shard_map import shard_map
        from concourse.bass2jax import (
            _bass_exec_p,
            install_neuronx_cc_hook,
            partition_id_tensor,
        )

        install_neuronx_cc_hook()
        self.jax = jax
        self.n_cores = n_cores
        partition_name = (
            nc.partition_id_tensor.name if nc.partition_id_tensor else None
        )
        in_names, out_names, out_avals, zero_shapes = [], [], [], []
        for alloc in nc.m.functions[0].allocations:
            if not isinstance(alloc, mybir.MemoryLocationSet):
                continue
            name = alloc.memorylocations[0].name
            if alloc.kind == "ExternalInput":
                if name != partition_name:
                    in_names.append(name)
            elif alloc.kind == "ExternalOutput":
                shape = tuple(alloc.tensor_shape)
                dtype = mybir.dt.np(alloc.dtype)
                out_names.append(name)
                out_avals.append(jax.core.ShapedArray(shape, dtype))
                zero_shapes.append((shape, dtype))
        self.in_names = in_names
        self.out_names = out_names
        self.out_avals = out_avals
        n_params = len(in_names)
        n_outs = len(out_names)
        all_names = list(in_names) + list(out_names)
        if partition_name is not None:
            all_names.append(partition_name)

        def _body(*args):
            operands = list(args)
            if partition_name is not None:
                operands.append(partition_id_tensor())
            outs = _bass_exec_p.bind(
                *operands,
                out_avals=tuple(out_avals),
                in_names=tuple(all_names),
                out_names=tuple(out_names),
                lowering_input_output_aliases=(),
                sim_require_finite=True,
                sim_require_nnan=True,
                nc=nc,
            )
            return tuple(outs)

        devices = jax.devices()[:n_cores]
        self.mesh = Mesh(_np.asarray(devices), ("core",))
        self.sharding = NamedSharding(self.mesh, PartitionSpec("core"))
        self.fn = jax.jit(
            shard_map(
                _body,
                mesh=self.mesh,
                in_specs=(PartitionSpec("core"),) * (n_params + n_outs),
                out_specs=(PartitionSpec("core"),) * n_outs,
                check_rep=False,
            ),
            donate_argnums=tuple(range(n_params, n_params + n_outs)),
            keep_unused=True,
        )
        self._zeros_fn = jax.jit(
            lambda: tuple(
                jax.numpy.zeros((n_cores * s[0], *s[1:]), d)
                for s, d in zero_shapes
            ),
            out_shardings=tuple(self.sharding for _ in zero_shapes),
        )

    def run(self, in_maps):
        jax = self.jax
        dev_inputs = [
            jax.device_put(
                np.concatenate(
                    [np.asarray(in_maps[c][n]) for c in range(self.n_cores)], axis=0
                ),
                self.sharding,
            )
            for n in self.in_names
        ]
        outs = self.fn(*dev_inputs, *self._zeros_fn())
        jax.block_until_ready(outs)
        res = []
        for c in range(self.n_cores):
            d = {}
            for i, name in enumerate(self.out_names):
                full = np.asarray(outs[i])
                d[name] = full.reshape(self.n_cores, *self.out_avals[i].shape)[c]
            res.append(d)
        return res


_RUNNER_CACHE = None


def _get_runner():
    global _RUNNER_CACHE
    if _RUNNER_CACHE is None:
        _RUNNER_CACHE = _Runner(_get_nc(), NCORES)
    return _RUNNER_CACHE


def kernel(x, lengths, emb, W_ih, W_hh, b_ih, b_hh):
    in_maps = make_in_maps(x, emb, W_ih, W_hh, b_ih, b_hh)
    res = _get_runner().run(in_maps)
    hidden = assemble_output(res, lengths)
    return hidden[:, -1], hidden
